# revision 6
# baseline (speedup 1.0000x reference)
"""Distributed Trainium2 Bass kernel for nn_Attention_79766132621772 (v3).

Reference computation (all fp32):
    B, L, D, H, HD = 2, 2048, 2048, 16, 128
    qkv = (x @ w_qkv).reshape(B, L, 3, H, HD)
    q, k = rope(q), rope(k)                       # positions along L
    att = softmax(q @ k^T / sqrt(HD))             # per (b, h)
    out = (att @ v).reshape(B, L, D) @ w_proj

Sharding: tensor-parallel over heads, 2 heads/core. Per-core partial
projections are summed with an on-device ReduceScatter (bf16 wire).

v3 dataflow — all bf16/fp16 (fp8 breaches the 2e-2 error budget: softmax
averaging does not suppress RELATIVE error, so each fp8 hop costs ~2.5e-2):
  stage 1: q/k/v bf16 matmuls; q/k evicted to bf16 with RoPE (rot = const
           bf16 matmul + 3 DVE ops); v evicted to fp16.
  stage 2: S bf16 -> exp on ScalarE (bias -2, cancels in softmax) -> fp16 P
           tiles; O = V^T P in fp16; the softmax denominator is an fp16
           pairwise-add tree on the DVE (2x 16-bit mode) plus ONE wide-ones
           matmul that replicates the row sums across all 128 partitions
           (no partition broadcast needed); normalize on PSUM eviction.
  stage 3: proj in bf16, chunked ReduceScatter(add) over 8 cores.
The emission is software-pipelined: the ScalarE exp stream gates stage 2, so
O/rowsum/projection/next-batch-QKV work is woven between exp slots at fine
grain to keep the in-order PE busy.
"""

import os
import ml_dtypes
import numpy as np

import concourse.bass as bass
import concourse.tile as tile
from concourse import bacc, mybir
from concourse._compat import axon_active
from concourse.bass_utils import run_bass_kernel_spmd

B, L, D, H = 2, 2048, 2048, 16
HD = 128
NCORES = 8
HPC = H // NCORES          # heads per core = 2
T = B * L                  # total tokens = 4096
TSHARD = T // NCORES       # output rows per core = 512
F32 = mybir.dt.float32
BF16 = mybir.dt.bfloat16
F16 = mybir.dt.float16
SCALE = 1.0 / float(np.sqrt(HD))
EXP_BIAS = -2.0            # max s ~ 6.1 -> max P = e^4.1 = 61 (cancels in softmax)

_CHUNK = 512               # q/token chunk width (moving dim of matmuls)
_NKT = D // 128            # 16 contraction tiles for D=2048
_NPR = _NKT // 2           # 8 DoubleRow contraction pairs
_NCH = L // _CHUNK         # 4 chunks per batch

# engine-assignment knobs (balance ACT/DVE/Pool load)
QK_EVICT = os.environ.get("QK_EVICT", "dve")    # act | dve
V_EVICT = os.environ.get("V_EVICT", "dve")      # act | dve
S3_EVICT = os.environ.get("S3_EVICT", "dve")    # mix | dve | act
ROPE_ENG = os.environ.get("ROPE_ENG", "dve")    # dve | pool (the 2 bf16 ops)
FCAST_ENG = os.environ.get("FCAST_ENG", "pool") # dve | pool
CHAIN_ENG = os.environ.get("CHAIN_ENG", "dve")  # mix | dve (rowsum chains)


def _build(reps=1, collective=True):
    nc = bacc.Bacc(
        "TRN2",
        target_bir_lowering=False,
        debug=not axon_active(),
        enable_asserts=False,
        num_devices=NCORES,
    )

    # ---- kernel I/O (per core) ----
    xT_d = nc.declare_dram_parameter("xT", [B, D, L], BF16, isOutput=False)
    wqkv_d = nc.declare_dram_parameter("w_qkv", [D, 6 * HD], BF16, isOutput=False)
    wproj_d = nc.declare_dram_parameter("w_proj", [HPC * HD, D], BF16, isOutput=False)
    cos_d = nc.declare_dram_parameter("cos", [HD, L], BF16, isOutput=False)
    sin_d = nc.declare_dram_parameter("sin", [HD, L], BF16, isOutput=False)
    out_d = nc.declare_dram_parameter("out", [TSHARD, D], BF16, isOutput=True)

    with tile.TileContext(nc) as tc:
        _emit(nc, tc, xT_d, wqkv_d, wproj_d, cos_d, sin_d, out_d, reps, collective)

    nc.compile()
    return nc


def _emit(nc, tc, xT_d, wqkv_d, wproj_d, cos_d, sin_d, out_d, reps=1, collective=True):
    fdma = nc.sync.dma_start
    NTT = L // 128            # token tiles per batch = 16
    NCC = int(os.environ.get("RS_CHUNKS", "4"))  # ReduceScatter chunk count
    RROWS = T // NCC // NCORES  # rows per rank per chunk = 128

    singles = tc.alloc_tile_pool(name="singles", bufs=1)
    # w_qkv in [128, kt, col] layout; cols: q_h0 q_h1 k_h0 k_h1 v_h0 v_h1
    w_sb = singles.tile([128, _NKT, 6 * HD], BF16)
    _wq_r = wqkv_d.ap().rearrange("(t p) c -> p t c", p=128)
    # first column block (q_h0 — the first accumulation) loads up front in 4
    # sub-pieces so the PE can start almost immediately; the remaining blocks
    # are emitted after the first x chunk's DMAs (see below), in ct
    # consumption order: k_h0 (2), q_h1 (1), k_h1 (3), then v (4, 5)
    for _g in range(4):
        fdma(
            out=w_sb[:, 4 * _g : 4 * _g + 4, 0:128],
            in_=_wq_r[:, 4 * _g : 4 * _g + 4, 0:128],
        )

    def load_w_rest():
        for _cb in (2, 1, 3, 4, 5):
            fdma(
                out=w_sb[:, :, _cb * 128 : (_cb + 1) * 128],
                in_=_wq_r[:, :, _cb * 128 : (_cb + 1) * 128],
            )
    # cos/sin/wproj go out on the Pool-engine DGE queue (and are emitted
    # after the first x chunk) so they don't contend with the
    # startup-critical w_qkv/x DMAs
    wproj_sb = singles.tile([128, HPC, D], BF16)
    cos_sb = singles.tile([HD, L], BF16)
    sin_sb = singles.tile([HD, L], BF16)

    def load_trig():
        nc.gpsimd.dma_start(out=cos_sb, in_=cos_d.ap())
        nc.gpsimd.dma_start(out=sin_sb, in_=sin_d.ap())
    # wide ones: the rowsum matmul replicates the per-column sum across all
    # 128 output partitions -> no partition broadcast before the normalize
    ones_f32 = singles.tile([128, 128], F32)
    nc.vector.memset(ones_f32, 1.0)
    ones16 = singles.tile([128, 128], F16)
    nc.vector.tensor_copy(out=ones16, in_=ones_f32)
    exp_bias = singles.tile([128, 1], F32)
    nc.vector.memset(exp_bias, EXP_BIAS)

    def load_wproj():
        nc.gpsimd.dma_start(
            out=wproj_sb, in_=wproj_d.ap().rearrange("(t p) c -> p t c", p=128)
        )

    # DRAM bounce buffers for the chunked collective (bf16 wire)
    dram = tc.alloc_tile_pool(name="dram", bufs=1, space="DRAM")
    bounce = [
        dram.tile([T // NCC, D], BF16, tag=f"bnc{i}", name=f"bounce_{i}")
        for i in range(NCC)
    ]
    rs_out = [
        dram.tile([RROWS, D], BF16, tag=f"rso{i}", name=f"rs_out_{i}")
        for i in range(NCC)
    ]

    # per-batch tiles; bufs=2 lets batch i+1's QKV overlap batch i's attention
    per_b = tc.alloc_tile_pool(name="per_b", bufs=int(os.environ.get("PB_BUFS", "2")))
    xp = tc.alloc_tile_pool(name="xp", bufs=int(os.environ.get("XG_BUFS", "16")))
    qs = tc.alloc_tile_pool(name="qs", bufs=int(os.environ.get("QS_BUFS", "3")))
    rp = tc.alloc_tile_pool(name="rp", bufs=int(os.environ.get("RP_BUFS", "2")))
    pp = tc.alloc_tile_pool(name="pp", bufs=int(os.environ.get("PP_BUFS", "9")))
    op = tc.alloc_tile_pool(name="op", bufs=int(os.environ.get("OT_BUFS", "3")))
    # PSUM budget (8 banks): S 2 + {proj, rot} ring 2 + qkv 2 + {po, pr} 2
    ps_s = tc.alloc_tile_pool(name="ps_s", bufs=2, space="PSUM")
    ps_p = tc.alloc_tile_pool(name="ps_p", bufs=2, space="PSUM")
    ps_qkv = tc.alloc_tile_pool(name="ps_qkv", bufs=2, space="PSUM")
    ps_o = tc.alloc_tile_pool(name="ps_o", bufs=2, space="PSUM")

    def rs_chunk(rep, cc):
        """ReduceScatter chunk cc, then one bf16 DMA into the output tensor
        (collectives may not write IO tensors directly)."""
        out_slice = out_d.ap()[cc * RROWS : (cc + 1) * RROWS, :]
        if collective:
            nc.gpsimd.collective_compute(
                "ReduceScatter",
                mybir.AluOpType.add,
                replica_groups=[list(range(NCORES))],
                ins=[bounce[cc].opt()],
                outs=[rs_out[cc].opt()],
            )
            fdma(out=out_slice, in_=rs_out[cc][:, :])
        else:
            fdma(out=out_slice, in_=bounce[cc][0:RROWS, :])

    NB = reps * B  # flattened batch index i = rep * B + b

    def batch_tiles(i):
        """per_b tiles for flattened batch i (pool ring depth handles reuse)."""
        qT_sb = per_b.tile([128, HPC, L], BF16, tag="qT", name=f"qT_{i}")
        kT_sb = per_b.tile([128, HPC, L], BF16, tag="kT", name=f"kT_{i}")
        v_sb = per_b.tile([128, NTT, HPC, HD], F16, tag="v", name=f"v_{i}")
        oT_sb = per_b.tile([128, HPC, L], BF16, tag="oT", name=f"oT_{i}")
        return qT_sb, kT_sb, v_sb, oT_sb

    tiles = {}

    def s1_thunks(i, ch):
        """QKV (fp8 DR) + RoPE thunks for one 512-token chunk of batch i."""
        qT_sb, kT_sb, v_sb, _ = tiles[i]
        b = i % B
        c0 = ch * _CHUNK
        xT_b = xT_d.ap()[b].rearrange("(t p) l -> p t l", p=128)  # [128,16,L]
        xg = []

        def dma_xg():
            for g in range(_NPR):
                xgt = xp.tile([128, 2, _CHUNK], BF16, tag="xg", name=f"xg_{i}_{ch}_{g}")
                fdma(out=xgt, in_=xT_b[:, 2 * g : 2 * g + 2, c0 : c0 + _CHUNK])
                xg.append(xgt)

        def qk_ct(ct):
            # ct order: q_h0, k_h0, q_h1, k_h1 — so head 0's S stream can
            # start as early as possible during warm-up
            h, is_k = divmod(ct, 2)
            dst = kT_sb if is_k else qT_sb
            wcol = (HPC + h if is_k else h) * 128
            pq = ps_qkv.tile([128, _CHUNK], F32, tag="pqk", name=f"pqk_{i}_{ch}_{ct}")
            for kt in range(_NKT):
                nc.tensor.matmul(
                    out=pq,
                    lhsT=w_sb[:, kt, wcol : wcol + 128],
                    rhs=xg[kt // 2][:, kt % 2, :],
                    start=(kt == 0),
                    stop=(kt == _NKT - 1),
                )
            # evict at natural scale to bf16, then rot = P_rot @ q via a
            # constant bf16 matmul
            qsb = qs.tile([128, _CHUNK], BF16, tag="qsb", name=f"qsb_{i}_{ch}_{ct}")
            if QK_EVICT == "act":
                nc.scalar.copy(out=qsb, in_=pq)
            else:
                nc.vector.tensor_copy(out=qsb, in_=pq)
            # rot(q) = pair swap (sign folded into the host-side sin): two
            # partition-strided SBUF->SBUF DMAs, no PE involvement
            qsw = qs.tile([128, _CHUNK], BF16, tag="qsw", name=f"qsw_{i}_{ch}_{ct}")
            fdma(out=qsw[0:128:2, :], in_=qsb[1:128:2, :])
            fdma(out=qsw[1:128:2, :], in_=qsb[0:128:2, :])
            # q' = q*cos + swap(q)*sin±  (all bf16 SBUF -> DVE 2x mode)
            cosc = cos_sb[:, c0 : c0 + _CHUNK]
            sinc = sin_sb[:, c0 : c0 + _CHUNK]
            dstc = dst[:, h, c0 : c0 + _CHUNK]
            tmp = rp.tile([128, _CHUNK], BF16, tag="rt", name=f"rt_{i}_{ch}_{ct}")
            eng2 = nc.gpsimd if ROPE_ENG == "pool" else nc.vector
            nc.vector.tensor_mul(out=tmp, in0=qsw, in1=sinc)
            eng2.tensor_mul(out=dstc, in0=qsb, in1=cosc)
            eng2.tensor_add(out=dstc, in0=dstc, in1=tmp)

        def v_tt(tt):
            pv = ps_qkv.tile([128, HPC * HD], F32, tag="pqk", name=f"pv_{i}_{ch}_{tt}")
            for kt in range(_NKT):
                nc.tensor.matmul(
                    out=pv,
                    lhsT=xg[kt // 2][:, kt % 2, tt * 128 : tt * 128 + 128],
                    rhs=w_sb[:, kt, 2 * HPC * 128 :],
                    start=(kt == 0),
                    stop=(kt == _NKT - 1),
                )
            gt = ch * (_CHUNK // 128) + tt
            v_dst = v_sb[:, gt, :, :].rearrange("p h d -> p (h d)")
            if V_EVICT == "act":
                nc.scalar.copy(out=v_dst, in_=pv)
            else:
                nc.vector.tensor_copy(out=v_dst, in_=pv)

        return (
            [dma_xg]
            + [lambda ct=ct: qk_ct(ct) for ct in range(2 * HPC)]
            + [lambda tt=tt: v_tt(tt) for tt in range(_CHUNK // 128)]
        )

    def se_thunks(i, h, qc):
        """16 (S-matmul, exp->fp8) thunks; returns (thunks, p8ts list)."""
        qT_sb, kT_sb, _, _ = tiles[i]
        q0 = qc * _CHUNK
        qT_c = qT_sb[:, h, q0 : q0 + _CHUNK]
        p8ts = []

        def se(kt):
            g, j = divmod(kt, 2)
            if j == 0:
                p8ts.append(
                    pp.tile([128, 2, _CHUNK], F16, tag="p8", name=f"p8_{i}_{h}_{qc}_{g}")
                )
            psS = ps_s.tile([128, _CHUNK], F32, tag="ps", name=f"ps_{i}_{h}_{qc}_{kt}")
            nc.tensor.matmul(
                out=psS,
                lhsT=kT_sb[:, h, kt * 128 : kt * 128 + 128],
                rhs=qT_c,
                start=True,
                stop=True,
            )
            nc.scalar.activation(
                out=p8ts[g][:, j, :], in_=psS,
                func=mybir.ActivationFunctionType.Exp,
                scale=SCALE, bias=exp_bias,
            )

        return [lambda kt=kt: se(kt) for kt in range(2 * _NPR)], p8ts

    def dr_thunks(i, h, qc, p8ts):
        """O matmuls (fp16), the DVE fp16 rowsum tree, and the normalize."""
        _, _, v_sb, oT_sb = tiles[i]
        q0 = qc * _CHUNK
        state = {}

        def o_mm(g):
            # two fp16 O matmuls consuming pair tile g
            if g == 0:
                state["po"] = ps_o.tile([128, _CHUNK], F32, tag="po", name=f"po_{i}_{h}_{qc}")
            for j in range(2):
                kt = 2 * g + j
                nc.tensor.matmul(
                    out=state["po"],
                    lhsT=v_sb[:, kt, h, :],
                    rhs=p8ts[g][:, j, :],
                    start=(kt == 0),
                    stop=(kt == 2 * _NPR - 1),
                )

        def chain(c, step):
            # 4 parallel fp16 accumulation chains (2 on DVE 2x, 2 on the
            # otherwise-idle Pool engine); bounded partial sums keep the
            # fp16 rounding error ~1e-4 of the total
            eng = nc.vector if c < 2 or CHAIN_ENG == "dve" else nc.gpsimd
            t = p8ts[2 * c + step // 2][:, step % 2, :]
            if step == 0:
                state[c] = rp.tile([128, _CHUNK], F16, tag=f"acc{c}", name=f"acc_{i}_{h}_{qc}_{c}")
                eng.tensor_add(out=state[c], in0=t, in1=p8ts[2 * c][:, 1, :])
            elif step >= 2:
                eng.tensor_add(out=state[c], in0=state[c], in1=t)

        def combine(n):
            if n < 2:
                nc.vector.tensor_add(
                    out=state[2 * n], in0=state[2 * n], in1=state[2 * n + 1]
                )
            else:
                nc.vector.tensor_add(out=state[0], in0=state[0], in1=state[2])
                # partition-reduce with a wide-ones matmul: pr = replicated sums
                state["pr"] = ps_o.tile([128, _CHUNK], F32, tag="po", name=f"pr_{i}_{h}_{qc}")
                nc.tensor.matmul(out=state["pr"], lhsT=ones16, rhs=state[0], start=True, stop=True)

        def norm():
            # O^T *= 1/rowsum (pr already replicated across partitions)
            rec = rp.tile([128, _CHUNK], F32, tag="rec", name=f"rec_{i}_{h}_{qc}")
            nc.vector.reciprocal(out=rec, in_=state["pr"])
            nc.vector.tensor_mul(
                out=oT_sb[:, h, q0 : q0 + _CHUNK], in0=state["po"], in1=rec
            )

        thunks = []
        for c in range(4):
            thunks.append(lambda c=c: o_mm(2 * c))
            thunks.append(lambda c=c: o_mm(2 * c + 1))
            thunks.append(lambda c=c: chain(c, 0))
            thunks.append(lambda c=c: chain(c, 2))
            thunks.append(lambda c=c: chain(c, 3))
        thunks += [lambda: combine(0), lambda: combine(1), lambda: combine(2), norm]
        return thunks

    def s3_thunks(i, qc):
        """Projection thunks for token tiles 4qc..4qc+3 (one per PSUM chunk)."""
        _, _, _, oT_sb = tiles[i]
        b = i % B
        thunks = []
        state = {}

        def mk(tt, nch):
            def run():
                if nch == 0:
                    state[tt] = op.tile([128, D], BF16, tag="ot", name=f"ot_{i}_{tt}")
                ot = state[tt]
                pout = ps_p.tile([128, _CHUNK], F32, tag="pp", name=f"pout_{i}_{tt}_{nch}")
                for h in range(HPC):
                    nc.tensor.matmul(
                        out=pout,
                        lhsT=oT_sb[:, h, tt * 128 : tt * 128 + 128],
                        rhs=wproj_sb[:, h, nch * _CHUNK : (nch + 1) * _CHUNK],
                        start=(h == 0),
                        stop=(h == HPC - 1),
                    )
                if S3_EVICT == "act" or (S3_EVICT == "mix" and nch % 2 == 0):
                    nc.scalar.copy(out=ot[:, nch * _CHUNK : (nch + 1) * _CHUNK], in_=pout)
                else:
                    nc.vector.tensor_copy(
                        out=ot[:, nch * _CHUNK : (nch + 1) * _CHUNK], in_=pout
                    )
                if nch == D // _CHUNK - 1:
                    cc = (b * NTT + tt) * NCC // (B * NTT)
                    row = (b * NTT + tt) * 128 - cc * (T // NCC)
                    fdma(out=bounce[cc][row : row + 128, :], in_=ot)
                    # chunk complete -> ReduceScatter it
                    if (b * NTT + tt + 1) % (B * NTT // NCC) == 0:
                        rs_chunk(i // B, cc)

            return run

        for tt in range(4 * qc, 4 * qc + 4):
            for nch in range(D // _CHUNK):
                thunks.append(mk(tt, nch))
        return thunks

    def weave(primary, fill):
        """Emit `primary` (ScalarE-gated SE stream) with `fill` (PE work)
        distributed evenly between its elements."""
        n = len(primary)
        done = 0
        for t, p in enumerate(primary):
            p()
            want = ((t + 1) * len(fill)) // n
            while done < want:
                fill[done]()
                done += 1
        while done < len(fill):
            fill[done]()
            done += 1

    # ---- software pipeline over flattened batches ----
    # Stage 2's S->exp stream is ScalarE-gated (exp ~3x the S matmul time) and
    # the PE runs in order, so PE work from the O/rowsum batch of the PREVIOUS
    # (h, qc) tuple, the projection of the previous qc group, and pending QKV
    # chunks is woven between SE slots at fine grain to fill the PE idle.
    # QKV chunks are emitted just-in-time from a queue: only chunk (0,0) runs
    # before attention starts; batch 0 steps emit two chunks each (catch-up),
    # later steps one (batch i+1's chunk, one step ahead of its deadline).
    from collections import deque

    tiles[0] = batch_tiles(0)
    chunkq = deque((i, ch) for i in range(NB) for ch in range(_NCH))
    th0 = s1_thunks(*chunkq.popleft())
    th0[0]()           # first x chunk's DMAs, right after w8's first block
    load_trig()        # cos/sin (Pool queue)
    load_w_rest()      # then the rest of w8
    load_wproj()       # w_proj last (needed ~40us in, at the first s3 group)
    for t in th0[1:]:
        t()
    carry = []  # thunks deferred to the next qc step (prev DR h1 + proj)
    for i in range(NB):
        if i + 1 < NB:
            tiles[i + 1] = batch_tiles(i + 1)
        for qc in range(_NCH):
            se0, p8_0 = se_thunks(i, 0, qc)
            se1, p8_1 = se_thunks(i, 1, qc)
            dr0 = dr_thunks(i, 0, qc, p8_0)
            dr1 = dr_thunks(i, 1, qc, p8_1)
            if i == 0 and qc == 0:
                # warm-up is PE-bound on QKV chunk production: make batch 0's
                # remaining chunks the primary stream and spread the (stall-
                # prone, ScalarE-gated) SE slots between them
                warm = []
                while chunkq and chunkq[0][0] == 0:
                    warm.extend(s1_thunks(*chunkq.popleft()))
                weave(warm, se0)
                fill_b = list(dr0)
                if chunkq and chunkq[0][0] <= 1:
                    fill_b.extend(s1_thunks(*chunkq.popleft()))
                weave(se1, fill_b)
            else:
                quota = 2 if i == 0 else 1
                fill_a, fill_b = list(carry), list(dr0)
                for n in range(quota):
                    if chunkq and chunkq[0][0] <= i + 1:
                        c = chunkq.popleft()
                        (fill_a if (i == 0 or n > 0) else fill_b).extend(s1_thunks(*c))
                weave(se0, fill_a)
                weave(se1, fill_b)
            carry = dr1 + s3_thunks(i, qc)
    for t in carry:
        t()

    for p in (ps_o, ps_qkv, ps_p, ps_s, op, pp, rp, qs, xp, per_b, dram, singles):
        p.release()


def _make_inputs(x, w_qkv, w_proj):
    x = np.asarray(x, dtype=np.float32)
    w_qkv = np.asarray(w_qkv, dtype=np.float32)
    w_proj = np.asarray(w_proj, dtype=np.float32)
    xT = np.ascontiguousarray(x.transpose(0, 2, 1)).astype(ml_dtypes.bfloat16)

    freqs = (1.0 / (10000.0 ** (np.arange(0, HD, 2, dtype=np.float32) / HD))).astype(
        np.float32
    )
    f = np.outer(np.arange(L, dtype=np.float32), freqs).astype(np.float32)  # [L, 64]
    cos_t = np.ascontiguousarray(
        np.repeat(np.cos(f), 2, axis=1).T.astype(ml_dtypes.bfloat16)
    )
    sin_t = np.ascontiguousarray(
        np.repeat(np.sin(f), 2, axis=1).T.astype(ml_dtypes.bfloat16)
    )

    # rot(q) is realized on-device as a partition pair swap; the signs of
    # the rotation fold into sin: row 2i gets -sin (multiplies q[2i+1]),
    # row 2i+1 keeps +sin (multiplies q[2i])
    sin_t = sin_t.astype(np.float32)
    sin_t[0::2, :] *= -1.0
    sin_t = sin_t.astype(ml_dtypes.bfloat16)

    in_maps = []
    for c in range(NCORES):
        heads = range(HPC * c, HPC * (c + 1))
        cols = []
        for s in (0, 1, 2):  # q, k, v columns for this core's heads
            for h in heads:
                cols.append(np.arange(s * D + h * HD, s * D + (h + 1) * HD))
        w_qkv_c = np.ascontiguousarray(
            w_qkv[:, np.concatenate(cols)].astype(ml_dtypes.bfloat16)
        )
        rows = np.concatenate([np.arange(h * HD, (h + 1) * HD) for h in heads])
        w_proj_c = np.ascontiguousarray(w_proj[rows, :].astype(ml_dtypes.bfloat16))
        in_maps.append(
            {
                "xT": xT,
                "w_qkv": w_qkv_c,
                "w_proj": w_proj_c,
                "cos": cos_t,
                "sin": sin_t,
            }
        )
    return in_maps


_NC_CACHE = None


def kernel(x, w_qkv, w_proj):
    global _NC_CACHE
    if _NC_CACHE is None:
        _NC_CACHE = _build()
    nc = _NC_CACHE
    in_maps = _make_inputs(x, w_qkv, w_proj)
    res = run_bass_kernel_spmd(nc, in_maps, core_ids=list(range(NCORES)))
    out = np.empty((T, D), dtype=np.float32)
    ncc = int(os.environ.get("RS_CHUNKS", "4"))
    rrows = T // ncc // NCORES
    for r in range(NCORES):
        o = res.results[r]["out"]
        for cc in range(ncc):
            out[cc * (T // ncc) + r * rrows : cc * (T // ncc) + (r + 1) * rrows] = o[
                cc * rrows : (cc + 1) * rrows
            ]
    return out.reshape(B, L, D).astype(np.float32)


# revision 7
# speedup vs baseline: 1.5279x; 1.5279x over previous
"""Distributed Trainium2 Bass kernel for nn_Attention_79766132621772 (v3).

Reference computation (all fp32):
    B, L, D, H, HD = 2, 2048, 2048, 16, 128
    qkv = (x @ w_qkv).reshape(B, L, 3, H, HD)
    q, k = rope(q), rope(k)                       # positions along L
    att = softmax(q @ k^T / sqrt(HD))             # per (b, h)
    out = (att @ v).reshape(B, L, D) @ w_proj

Sharding: tensor-parallel over heads, 2 heads/core. Per-core partial
projections are summed with an on-device ReduceScatter (bf16 wire).

v3 dataflow — all bf16/fp16 (fp8 breaches the 2e-2 error budget: softmax
averaging does not suppress RELATIVE error, so each fp8 hop costs ~2.5e-2):
  stage 1: q/k/v bf16 matmuls; q/k evicted to bf16 with RoPE (rot = const
           bf16 matmul + 3 DVE ops); v evicted to fp16.
  stage 2: S bf16 -> exp on ScalarE (bias -2, cancels in softmax) -> fp16 P
           tiles; O = V^T P in fp16; the softmax denominator is an fp16
           pairwise-add tree on the DVE (2x 16-bit mode) plus ONE wide-ones
           matmul that replicates the row sums across all 128 partitions
           (no partition broadcast needed); normalize on PSUM eviction.
  stage 3: proj in bf16, chunked ReduceScatter(add) over 8 cores.
The emission is software-pipelined: the ScalarE exp stream gates stage 2, so
O/rowsum/projection/next-batch-QKV work is woven between exp slots at fine
grain to keep the in-order PE busy.
"""

import os
import ml_dtypes
import numpy as np

import concourse.bass as bass
import concourse.tile as tile
from concourse import bacc, mybir
from concourse._compat import axon_active
from concourse.bass_utils import run_bass_kernel_spmd

B, L, D, H = 2, 2048, 2048, 16
HD = 128
NCORES = 8
HPC = H // NCORES          # heads per core = 2
T = B * L                  # total tokens = 4096
TSHARD = T // NCORES       # output rows per core = 512
F32 = mybir.dt.float32
BF16 = mybir.dt.bfloat16
F16 = mybir.dt.float16
SCALE = 1.0 / float(np.sqrt(HD))
EXP_BIAS = -2.0            # max s ~ 6.1 -> max P = e^4.1 = 61 (cancels in softmax)

_CHUNK = 512               # q/token chunk width (moving dim of matmuls)
_NKT = D // 128            # 16 contraction tiles for D=2048
_NPR = _NKT // 2           # 8 DoubleRow contraction pairs
_NCH = L // _CHUNK         # 4 chunks per batch

# engine-assignment knobs (balance ACT/DVE/Pool load)
QK_EVICT = os.environ.get("QK_EVICT", "dve")    # act | dve
V_EVICT = os.environ.get("V_EVICT", "dve")      # act | dve
S3_EVICT = os.environ.get("S3_EVICT", "dve")    # mix | dve | act
ROPE_ENG = os.environ.get("ROPE_ENG", "dve")    # dve | pool (the 2 bf16 ops)
FCAST_ENG = os.environ.get("FCAST_ENG", "pool") # dve | pool
CHAIN_ENG = os.environ.get("CHAIN_ENG", "dve")  # mix | dve (rowsum chains)


def _build(reps=1, collective=True):
    nc = bacc.Bacc(
        "TRN2",
        target_bir_lowering=False,
        debug=not axon_active(),
        enable_asserts=False,
        num_devices=NCORES,
    )

    # ---- kernel I/O (per core) ----
    xT_d = nc.declare_dram_parameter("xT", [B, D, L], BF16, isOutput=False)
    wqkv_d = nc.declare_dram_parameter("w_qkv", [D, 6 * HD], BF16, isOutput=False)
    wproj_d = nc.declare_dram_parameter("w_proj", [HPC * HD, D], BF16, isOutput=False)
    cos_d = nc.declare_dram_parameter("cos", [HD, L], BF16, isOutput=False)
    sin_d = nc.declare_dram_parameter("sin", [HD, L], BF16, isOutput=False)
    out_d = nc.declare_dram_parameter("out", [TSHARD, D], BF16, isOutput=True)

    with tile.TileContext(nc) as tc:
        _emit(nc, tc, xT_d, wqkv_d, wproj_d, cos_d, sin_d, out_d, reps, collective)

    nc.compile()
    return nc


def _emit(nc, tc, xT_d, wqkv_d, wproj_d, cos_d, sin_d, out_d, reps=1, collective=True):
    fdma = nc.sync.dma_start
    NTT = L // 128            # token tiles per batch = 16
    NCC = int(os.environ.get("RS_CHUNKS", "4"))  # ReduceScatter chunk count
    RROWS = T // NCC // NCORES  # rows per rank per chunk = 128

    singles = tc.alloc_tile_pool(name="singles", bufs=1)
    # w_qkv in [128, kt, col] layout; cols: q_h0 q_h1 k_h0 k_h1 v_h0 v_h1
    w_sb = singles.tile([128, _NKT, 6 * HD], BF16)
    _wq_r = wqkv_d.ap().rearrange("(t p) c -> p t c", p=128)
    # first column block (q_h0 — the first accumulation) loads up front in 4
    # sub-pieces so the PE can start almost immediately; the remaining blocks
    # are emitted after the first x chunk's DMAs (see below), in ct
    # consumption order: k_h0 (2), q_h1 (1), k_h1 (3), then v (4, 5)
    for _g in range(4):
        fdma(
            out=w_sb[:, 4 * _g : 4 * _g + 4, 0:128],
            in_=_wq_r[:, 4 * _g : 4 * _g + 4, 0:128],
        )

    def load_w_rest():
        for _cb in (2, 1, 3, 4, 5):
            fdma(
                out=w_sb[:, :, _cb * 128 : (_cb + 1) * 128],
                in_=_wq_r[:, :, _cb * 128 : (_cb + 1) * 128],
            )
    # cos/sin/wproj go out on the Pool-engine DGE queue (and are emitted
    # after the first x chunk) so they don't contend with the
    # startup-critical w_qkv/x DMAs
    wproj_sb = singles.tile([128, HPC, D], BF16)
    cos_sb = singles.tile([HD, L], BF16)
    sin_sb = singles.tile([HD, L], BF16)

    def load_trig():
        nc.gpsimd.dma_start(out=cos_sb, in_=cos_d.ap())
        nc.gpsimd.dma_start(out=sin_sb, in_=sin_d.ap())
    # wide ones: the rowsum matmul replicates the per-column sum across all
    # 128 output partitions -> no partition broadcast before the normalize
    ones_f32 = singles.tile([128, 128], F32)
    nc.vector.memset(ones_f32, 1.0)
    ones16 = singles.tile([128, 128], F16)
    nc.vector.tensor_copy(out=ones16, in_=ones_f32)
    exp_bias = singles.tile([128, 1], F32)
    nc.vector.memset(exp_bias, EXP_BIAS)

    def load_wproj():
        nc.gpsimd.dma_start(
            out=wproj_sb, in_=wproj_d.ap().rearrange("(t p) c -> p t c", p=128)
        )

    # DRAM bounce buffers for the chunked collective (bf16 wire)
    dram = tc.alloc_tile_pool(name="dram", bufs=1, space="DRAM")
    bounce = [
        dram.tile([T // NCC, D], BF16, tag=f"bnc{i}", name=f"bounce_{i}")
        for i in range(NCC)
    ]
    rs_out = [
        dram.tile([RROWS, D], BF16, tag=f"rso{i}", name=f"rs_out_{i}")
        for i in range(NCC)
    ]

    # per-batch tiles; bufs=2 lets batch i+1's QKV overlap batch i's attention
    per_b = tc.alloc_tile_pool(name="per_b", bufs=int(os.environ.get("PB_BUFS", "2")))
    xp = tc.alloc_tile_pool(name="xp", bufs=int(os.environ.get("XG_BUFS", "16")))
    qs = tc.alloc_tile_pool(name="qs", bufs=int(os.environ.get("QS_BUFS", "3")))
    rp = tc.alloc_tile_pool(name="rp", bufs=int(os.environ.get("RP_BUFS", "2")))
    pp = tc.alloc_tile_pool(name="pp", bufs=int(os.environ.get("PP_BUFS", "9")))
    op = tc.alloc_tile_pool(name="op", bufs=int(os.environ.get("OT_BUFS", "3")))
    # PSUM budget (8 banks): S 2 + {proj, rot} ring 2 + qkv 2 + {po, pr} 2
    ps_s = tc.alloc_tile_pool(name="ps_s", bufs=2, space="PSUM")
    ps_p = tc.alloc_tile_pool(name="ps_p", bufs=2, space="PSUM")
    ps_qkv = tc.alloc_tile_pool(name="ps_qkv", bufs=2, space="PSUM")
    ps_o = tc.alloc_tile_pool(name="ps_o", bufs=2, space="PSUM")

    def rs_chunk(rep, cc):
        """ReduceScatter chunk cc, then one bf16 DMA into the output tensor
        (collectives may not write IO tensors directly)."""
        out_slice = out_d.ap()[cc * RROWS : (cc + 1) * RROWS, :]
        if collective:
            nc.gpsimd.collective_compute(
                "ReduceScatter",
                mybir.AluOpType.add,
                replica_groups=[list(range(NCORES))],
                ins=[bounce[cc].opt()],
                outs=[rs_out[cc].opt()],
            )
            fdma(out=out_slice, in_=rs_out[cc][:, :])
        else:
            fdma(out=out_slice, in_=bounce[cc][0:RROWS, :])

    NB = reps * B  # flattened batch index i = rep * B + b

    def batch_tiles(i):
        """per_b tiles for flattened batch i (pool ring depth handles reuse)."""
        qT_sb = per_b.tile([128, HPC, L], BF16, tag="qT", name=f"qT_{i}")
        kT_sb = per_b.tile([128, HPC, L], BF16, tag="kT", name=f"kT_{i}")
        v_sb = per_b.tile([128, NTT, HPC, HD], F16, tag="v", name=f"v_{i}")
        oT_sb = per_b.tile([128, HPC, L], BF16, tag="oT", name=f"oT_{i}")
        return qT_sb, kT_sb, v_sb, oT_sb

    tiles = {}

    def s1_thunks(i, ch):
        """QKV (fp8 DR) + RoPE thunks for one 512-token chunk of batch i."""
        qT_sb, kT_sb, v_sb, _ = tiles[i]
        b = i % B
        c0 = ch * _CHUNK
        xT_b = xT_d.ap()[b].rearrange("(t p) l -> p t l", p=128)  # [128,16,L]
        xg = []

        def dma_xg():
            for g in range(_NPR):
                xgt = xp.tile([128, 2, _CHUNK], BF16, tag="xg", name=f"xg_{i}_{ch}_{g}")
                fdma(out=xgt, in_=xT_b[:, 2 * g : 2 * g + 2, c0 : c0 + _CHUNK])
                xg.append(xgt)

        def qk_ct(ct):
            # ct order: q_h0, k_h0, q_h1, k_h1 — so head 0's S stream can
            # start as early as possible during warm-up
            h, is_k = divmod(ct, 2)
            dst = kT_sb if is_k else qT_sb
            wcol = (HPC + h if is_k else h) * 128
            pq = ps_qkv.tile([128, _CHUNK], F32, tag="pqk", name=f"pqk_{i}_{ch}_{ct}")
            for kt in range(_NKT):
                nc.tensor.matmul(
                    out=pq,
                    lhsT=w_sb[:, kt, wcol : wcol + 128],
                    rhs=xg[kt // 2][:, kt % 2, :],
                    start=(kt == 0),
                    stop=(kt == _NKT - 1),
                )
            # evict at natural scale to bf16, then rot = P_rot @ q via a
            # constant bf16 matmul
            qsb = qs.tile([128, _CHUNK], BF16, tag="qsb", name=f"qsb_{i}_{ch}_{ct}")
            if QK_EVICT == "act":
                nc.scalar.copy(out=qsb, in_=pq)
            else:
                nc.vector.tensor_copy(out=qsb, in_=pq)
            # rot(q) = pair swap (sign folded into the host-side sin): two
            # partition-strided SBUF->SBUF DMAs, no PE involvement
            qsw = qs.tile([128, _CHUNK], BF16, tag="qsw", name=f"qsw_{i}_{ch}_{ct}")
            fdma(out=qsw[0:128:2, :], in_=qsb[1:128:2, :])
            fdma(out=qsw[1:128:2, :], in_=qsb[0:128:2, :])
            # q' = q*cos + swap(q)*sin±  (all bf16 SBUF -> DVE 2x mode)
            cosc = cos_sb[:, c0 : c0 + _CHUNK]
            sinc = sin_sb[:, c0 : c0 + _CHUNK]
            dstc = dst[:, h, c0 : c0 + _CHUNK]
            tmp = rp.tile([128, _CHUNK], BF16, tag="rt", name=f"rt_{i}_{ch}_{ct}")
            eng2 = nc.gpsimd if ROPE_ENG == "pool" else nc.vector
            nc.vector.tensor_mul(out=tmp, in0=qsw, in1=sinc)
            eng2.tensor_mul(out=dstc, in0=qsb, in1=cosc)
            eng2.tensor_add(out=dstc, in0=dstc, in1=tmp)

        def v_tt(tt):
            pv = ps_qkv.tile([128, HPC * HD], F32, tag="pqk", name=f"pv_{i}_{ch}_{tt}")
            for kt in range(_NKT):
                nc.tensor.matmul(
                    out=pv,
                    lhsT=xg[kt // 2][:, kt % 2, tt * 128 : tt * 128 + 128],
                    rhs=w_sb[:, kt, 2 * HPC * 128 :],
                    start=(kt == 0),
                    stop=(kt == _NKT - 1),
                )
            gt = ch * (_CHUNK // 128) + tt
            v_dst = v_sb[:, gt, :, :].rearrange("p h d -> p (h d)")
            if V_EVICT == "act":
                nc.scalar.copy(out=v_dst, in_=pv)
            else:
                nc.vector.tensor_copy(out=v_dst, in_=pv)

        return (
            [dma_xg]
            + [lambda ct=ct: qk_ct(ct) for ct in range(2 * HPC)]
            + [lambda tt=tt: v_tt(tt) for tt in range(_CHUNK // 128)]
        )

    def se_thunks(i, h, qc):
        """16 (S-matmul, exp->fp8) thunks; returns (thunks, p8ts list)."""
        qT_sb, kT_sb, _, _ = tiles[i]
        q0 = qc * _CHUNK
        qT_c = qT_sb[:, h, q0 : q0 + _CHUNK]
        p8ts = []

        def se(kt):
            g, j = divmod(kt, 2)
            if j == 0:
                p8ts.append(
                    pp.tile([128, 2, _CHUNK], F16, tag="p8", name=f"p8_{i}_{h}_{qc}_{g}")
                )
            psS = ps_s.tile([128, _CHUNK], F32, tag="ps", name=f"ps_{i}_{h}_{qc}_{kt}")
            nc.tensor.matmul(
                out=psS,
                lhsT=kT_sb[:, h, kt * 128 : kt * 128 + 128],
                rhs=qT_c,
                start=True,
                stop=True,
            )
            nc.scalar.activation(
                out=p8ts[g][:, j, :], in_=psS,
                func=mybir.ActivationFunctionType.Exp,
                scale=SCALE, bias=exp_bias,
            )

        return [lambda kt=kt: se(kt) for kt in range(2 * _NPR)], p8ts

    def dr_thunks(i, h, qc, p8ts):
        """O matmuls (fp16), the DVE fp16 rowsum tree, and the normalize."""
        _, _, v_sb, oT_sb = tiles[i]
        q0 = qc * _CHUNK
        state = {}

        def o_mm(g):
            # two fp16 O matmuls consuming pair tile g
            if g == 0:
                state["po"] = ps_o.tile([128, _CHUNK], F32, tag="po", name=f"po_{i}_{h}_{qc}")
            for j in range(2):
                kt = 2 * g + j
                nc.tensor.matmul(
                    out=state["po"],
                    lhsT=v_sb[:, kt, h, :],
                    rhs=p8ts[g][:, j, :],
                    start=(kt == 0),
                    stop=(kt == 2 * _NPR - 1),
                )

        def chain(c, step):
            # 4 parallel fp16 accumulation chains (2 on DVE 2x, 2 on the
            # otherwise-idle Pool engine); bounded partial sums keep the
            # fp16 rounding error ~1e-4 of the total
            eng = nc.vector if c < 2 or CHAIN_ENG == "dve" else nc.gpsimd
            t = p8ts[2 * c + step // 2][:, step % 2, :]
            if step == 0:
                state[c] = rp.tile([128, _CHUNK], F16, tag=f"acc{c}", name=f"acc_{i}_{h}_{qc}_{c}")
                eng.tensor_add(out=state[c], in0=t, in1=p8ts[2 * c][:, 1, :])
            elif step >= 2:
                eng.tensor_add(out=state[c], in0=state[c], in1=t)

        def combine(n):
            if n < 2:
                nc.vector.tensor_add(
                    out=state[2 * n], in0=state[2 * n], in1=state[2 * n + 1]
                )
            else:
                nc.vector.tensor_add(out=state[0], in0=state[0], in1=state[2])
                # partition-reduce with a wide-ones matmul: pr = replicated sums
                state["pr"] = ps_o.tile([128, _CHUNK], F32, tag="po", name=f"pr_{i}_{h}_{qc}")
                nc.tensor.matmul(out=state["pr"], lhsT=ones16, rhs=state[0], start=True, stop=True)

        def norm():
            # O^T *= 1/rowsum (pr already replicated across partitions)
            rec = rp.tile([128, _CHUNK], F32, tag="rec", name=f"rec_{i}_{h}_{qc}")
            nc.vector.reciprocal(out=rec, in_=state["pr"])
            nc.vector.tensor_mul(
                out=oT_sb[:, h, q0 : q0 + _CHUNK], in0=state["po"], in1=rec
            )

        thunks = []
        for c in range(4):
            thunks.append(lambda c=c: o_mm(2 * c))
            thunks.append(lambda c=c: o_mm(2 * c + 1))
            thunks.append(lambda c=c: chain(c, 0))
            thunks.append(lambda c=c: chain(c, 2))
            thunks.append(lambda c=c: chain(c, 3))
        thunks += [lambda: combine(0), lambda: combine(1), lambda: combine(2), norm]
        return thunks

    def s3_thunks(i, qc, tail=False):
        """Projection thunks for token tiles 4qc..4qc+3 (one per PSUM chunk)."""
        _, _, _, oT_sb = tiles[i]
        b = i % B
        thunks = []
        state = {}

        def mk(tt, nch):
            def run():
                if nch == 0:
                    state[tt] = op.tile([128, D], BF16, tag="ot", name=f"ot_{i}_{tt}")
                ot = state[tt]
                pout = ps_p.tile([128, _CHUNK], F32, tag="pp", name=f"pout_{i}_{tt}_{nch}")
                for h in range(HPC):
                    nc.tensor.matmul(
                        out=pout,
                        lhsT=oT_sb[:, h, tt * 128 : tt * 128 + 128],
                        rhs=wproj_sb[:, h, nch * _CHUNK : (nch + 1) * _CHUNK],
                        start=(h == 0),
                        stop=(h == HPC - 1),
                    )
                if tail or S3_EVICT == "act" or (S3_EVICT == "mix" and nch % 2 == 0):
                    nc.scalar.copy(out=ot[:, nch * _CHUNK : (nch + 1) * _CHUNK], in_=pout)
                else:
                    nc.vector.tensor_copy(
                        out=ot[:, nch * _CHUNK : (nch + 1) * _CHUNK], in_=pout
                    )
                if nch == D // _CHUNK - 1:
                    cc = (b * NTT + tt) * NCC // (B * NTT)
                    row = (b * NTT + tt) * 128 - cc * (T // NCC)
                    fdma(out=bounce[cc][row : row + 128, :], in_=ot)
                    # chunk complete -> ReduceScatter it
                    if (b * NTT + tt + 1) % (B * NTT // NCC) == 0:
                        rs_chunk(i // B, cc)

            return run

        for tt in range(4 * qc, 4 * qc + 4):
            for nch in range(D // _CHUNK):
                thunks.append(mk(tt, nch))
        return thunks

    def weave(primary, fill):
        """Emit `primary` (ScalarE-gated SE stream) with `fill` (PE work)
        distributed evenly between its elements."""
        n = len(primary)
        done = 0
        for t, p in enumerate(primary):
            p()
            want = ((t + 1) * len(fill)) // n
            while done < want:
                fill[done]()
                done += 1
        while done < len(fill):
            fill[done]()
            done += 1

    # ---- software pipeline over flattened batches ----
    # Stage 2's S->exp stream is ScalarE-gated (exp ~3x the S matmul time) and
    # the PE runs in order, so PE work from the O/rowsum batch of the PREVIOUS
    # (h, qc) tuple, the projection of the previous qc group, and pending QKV
    # chunks is woven between SE slots at fine grain to fill the PE idle.
    # QKV chunks are emitted just-in-time from a queue: only chunk (0,0) runs
    # before attention starts; batch 0 steps emit two chunks each (catch-up),
    # later steps one (batch i+1's chunk, one step ahead of its deadline).
    from collections import deque

    tiles[0] = batch_tiles(0)
    chunkq = deque((i, ch) for i in range(NB) for ch in range(_NCH))
    th0 = s1_thunks(*chunkq.popleft())
    th0[0]()           # first x chunk's DMAs, right after w8's first block
    load_trig()        # cos/sin (Pool queue)
    load_w_rest()      # then the rest of w8
    load_wproj()       # w_proj last (needed ~40us in, at the first s3 group)
    for t in th0[1:]:
        t()
    carry = []  # thunks deferred to the next qc step (prev DR h1 + proj)
    for i in range(NB):
        if i + 1 < NB:
            tiles[i + 1] = batch_tiles(i + 1)
        for qc in range(_NCH):
            se0, p8_0 = se_thunks(i, 0, qc)
            se1, p8_1 = se_thunks(i, 1, qc)
            dr0 = dr_thunks(i, 0, qc, p8_0)
            dr1 = dr_thunks(i, 1, qc, p8_1)
            if i == 0 and qc == 0:
                # warm-up is PE-bound on QKV chunk production: make batch 0's
                # remaining chunks the primary stream and spread the (stall-
                # prone, ScalarE-gated) SE slots between them
                warm = []
                while chunkq and chunkq[0][0] == 0:
                    warm.extend(s1_thunks(*chunkq.popleft()))
                weave(warm, se0)
                fill_b = list(dr0)
                if chunkq and chunkq[0][0] <= 1:
                    fill_b.extend(s1_thunks(*chunkq.popleft()))
                weave(se1, fill_b)
            else:
                quota = 2 if i == 0 else 1
                fill_a, fill_b = list(carry), list(dr0)
                for n in range(quota):
                    if chunkq and chunkq[0][0] <= i + 1:
                        c = chunkq.popleft()
                        (fill_a if (i == 0 or n > 0) else fill_b).extend(s1_thunks(*c))
                weave(se0, fill_a)
                weave(se1, fill_b)
            carry = dr1 + s3_thunks(i, qc, tail=(i == NB - 1 and qc == _NCH - 1))
    for t in carry:
        t()

    for p in (ps_o, ps_qkv, ps_p, ps_s, op, pp, rp, qs, xp, per_b, dram, singles):
        p.release()


def _make_inputs(x, w_qkv, w_proj):
    x = np.asarray(x, dtype=np.float32)
    w_qkv = np.asarray(w_qkv, dtype=np.float32)
    w_proj = np.asarray(w_proj, dtype=np.float32)
    xT = np.ascontiguousarray(x.transpose(0, 2, 1)).astype(ml_dtypes.bfloat16)

    freqs = (1.0 / (10000.0 ** (np.arange(0, HD, 2, dtype=np.float32) / HD))).astype(
        np.float32
    )
    f = np.outer(np.arange(L, dtype=np.float32), freqs).astype(np.float32)  # [L, 64]
    cos_t = np.ascontiguousarray(
        np.repeat(np.cos(f), 2, axis=1).T.astype(ml_dtypes.bfloat16)
    )
    sin_t = np.ascontiguousarray(
        np.repeat(np.sin(f), 2, axis=1).T.astype(ml_dtypes.bfloat16)
    )

    # rot(q) is realized on-device as a partition pair swap; the signs of
    # the rotation fold into sin: row 2i gets -sin (multiplies q[2i+1]),
    # row 2i+1 keeps +sin (multiplies q[2i])
    sin_t = sin_t.astype(np.float32)
    sin_t[0::2, :] *= -1.0
    sin_t = sin_t.astype(ml_dtypes.bfloat16)

    in_maps = []
    for c in range(NCORES):
        heads = range(HPC * c, HPC * (c + 1))
        cols = []
        for s in (0, 1, 2):  # q, k, v columns for this core's heads
            for h in heads:
                cols.append(np.arange(s * D + h * HD, s * D + (h + 1) * HD))
        w_qkv_c = np.ascontiguousarray(
            w_qkv[:, np.concatenate(cols)].astype(ml_dtypes.bfloat16)
        )
        rows = np.concatenate([np.arange(h * HD, (h + 1) * HD) for h in heads])
        w_proj_c = np.ascontiguousarray(w_proj[rows, :].astype(ml_dtypes.bfloat16))
        in_maps.append(
            {
                "xT": xT,
                "w_qkv": w_qkv_c,
                "w_proj": w_proj_c,
                "cos": cos_t,
                "sin": sin_t,
            }
        )
    return in_maps


_NC_CACHE = None


def kernel(x, w_qkv, w_proj):
    global _NC_CACHE
    if _NC_CACHE is None:
        _NC_CACHE = _build()
    nc = _NC_CACHE
    in_maps = _make_inputs(x, w_qkv, w_proj)
    res = run_bass_kernel_spmd(nc, in_maps, core_ids=list(range(NCORES)))
    out = np.empty((T, D), dtype=np.float32)
    ncc = int(os.environ.get("RS_CHUNKS", "4"))
    rrows = T // ncc // NCORES
    for r in range(NCORES):
        o = res.results[r]["out"]
        for cc in range(ncc):
            out[cc * (T // ncc) + r * rrows : cc * (T // ncc) + (r + 1) * rrows] = o[
                cc * rrows : (cc + 1) * rrows
            ]
    return out.reshape(B, L, D).astype(np.float32)
